# revision 1
# baseline (speedup 1.0000x reference)
"""Trainium2 Bass kernel for nn_CrossAttention (single-query cross attention).

Reference computation (B=4, C=64, H=W=128, heads h=64, dim_head d=64,
inner=4096, HW=16384):
    x[b, j, c]   = fimg[b, c, j]                       (j indexes H*W)
    q[b, h, d]   = sum_e fpsf[b, e] Wq[h*64+d, e]
    k[b, j, h, d]= sum_c x[b, j, c] Wk[h*64+d, c]
    out[b, h, j] = scale * sum_d q[b,h,d] k[b,j,h,d]

Because there is a single query per (batch, head), the attention collapses:
    W2[b, h, c]  = scale * sum_d q[b,h,d] Wk[h*64+d, c]      (tiny)
    out[b, h, j] = sum_c W2[b,h,c] fimg[b, c, j]
a 64x FLOP reduction vs materializing k. The kernel is then bound by
per-instruction TensorE overhead (64 small matmuls) and DMA.

Sharding: the j (H*W = 16384) axis is split across the 8 cores (2048 each).
Every core redundantly computes W2 (it needs all heads for its output).

dtypes: weights and fimg are converted to bf16 on the host (layout prep);
matmuls run bf16 -> f32 PSUM; output is f32. rel err ~3e-3 « 2e-2 gate.

Device layouts (prepared host-side; host does LAYOUT only, no math):
  WqF    [64, 4100] bf16: cols 0:4 = fpsf.T, cols 4: = Wq.T  (packed into
                         one tensor so a single efficient DMA delivers the
                         whole A-stage input; kept at base partition 0:
                         bf16 matmuls with operands at partition offset 64
                         crash TRN2)
  Wk_bd  [128, 4096] bf16: per head-pair p, cols 128p..128p+128 hold
                         block-diag [[Wk_{2p}[d,c], 0], [0, Wk_{2p+1}[d,c]]]
  fimg_s [256, 2048] bf16: rows b*64+c, cols = local j
  out    [256, 2048] f32 : rows b*64+h, cols = local j

Device compute per core:
  A: 32 matmuls  q2T chunk [128, 4] = WqT_chunk.T @ fpsfT
     -> q2T psum [128, 128] with cols 4p+b
  copy: q2T psum -> SBUF bf16 with the attention scale folded in
  B: 32 matmuls  w2 [128, 4] = Wk_bd_p.T @ q2T[:, 4p:4p+4]
     -> w2 psum [128, 128]: rows c + 64*(h%2), cols 4*(h//2)+b
  Assembly: per batch-pair q, block-diag lhsT bd_q [128, 128] (bf16):
     bd_q[64*half + c, 64*half + h] = W2[2q+half, h, c]
  Big: 8 matmuls [128, 512] = bd_q.T @ fimg rows-pair; psum -> f32 SBUF
     staging [128, 1024] (vector/scalar alternate); 4 output DMAs.

Measured on TRN2 (8 cores, axon): ~28-30us NEFF exec, rel err 3.8e-3.
A trivial copy kernel measures ~15.6us on this stack (NEFF fixed
overhead: entry barriers + engine program loads + exit drain), so the
marginal cost of the whole computation is ~13us.
"""

import sys
import types

import numpy as np
import ml_dtypes

# antenv.axon_hooks is absent in this image; bass_utils imports it when
# tracing. Register a minimal stand-in before importing concourse.
if "antenv.axon_hooks" not in sys.modules:
    try:
        import antenv  # noqa: F401

        _hooks = types.ModuleType("antenv.axon_hooks")
        _hooks._hook = None

        def _set_hook(h):
            _hooks._hook = h

        _hooks.set_axon_ntff_profile_hook = _set_hook
        _hooks.get_axon_ntff_profile_hook = lambda: _hooks._hook
        sys.modules["antenv.axon_hooks"] = _hooks
        try:
            from trn_agent_boot.trn_boot import _ntff_profile_via_ctypes

            _set_hook(_ntff_profile_via_ctypes("/opt/axon/libaxon_pjrt.so"))
        except Exception:
            pass
    except ImportError:
        pass

import concourse.bass as bass  # noqa: E402
import concourse.mybir as mybir  # noqa: E402
import concourse.tile as tile  # noqa: E402
from concourse import bacc  # noqa: E402
from concourse.bass_utils import run_bass_kernel_spmd  # noqa: E402

N_CORES = 8
B, C, H, W = 4, 64, 128, 128
HEADS, DIM_HEAD = 64, 64
HW = H * W
JS = HW // N_CORES  # 2048 j-positions per core
SCALE = DIM_HEAD ** -0.5
F32 = mybir.dt.float32
BF16 = mybir.dt.bfloat16
NPBF16 = ml_dtypes.bfloat16

_compiled = None  # cache (nc) across calls


def _build():
    nc = bacc.Bacc("TRN2", target_bir_lowering=False, debug=False,
                   num_devices=N_CORES)

    fimg_d = nc.dram_tensor("fimg_s", [2 * 128, JS], BF16, kind="ExternalInput")
    wqf_d = nc.dram_tensor("WqF", [64, 4100], BF16, kind="ExternalInput")
    wkbd_d = nc.dram_tensor("Wk_bd", [128, 4096], BF16, kind="ExternalInput")
    out_d = nc.dram_tensor("out", [2 * 128, JS], F32, kind="ExternalOutput")

    with tile.TileContext(nc) as tc:
        with (
            tc.tile_pool(name="weights", bufs=1) as wpool,
            tc.tile_pool(name="img", bufs=1) as ipool,
            tc.tile_pool(name="small_ps", bufs=1, space="PSUM") as spsum,
            tc.tile_pool(name="big_ps", bufs=6, space="PSUM") as bpsum,
            tc.tile_pool(name="ostage", bufs=8) as opool,
        ):
            # fpsf.T and Wq.T packed in one [64, 4100] image, DMA'd in
            # two halves so step-A matmuls on early chunks can start
            # before the whole tensor lands.
            wqf = wpool.tile([64, 4100], BF16, tag="wqf")
            nc.sync.dma_start(wqf[:, 0:1028], wqf_d.ap()[:, 0:1028])
            nc.sync.dma_start(wqf[:, 1028:2052], wqf_d.ap()[:, 1028:2052])
            nc.sync.dma_start(wqf[:, 2052:4100], wqf_d.ap()[:, 2052:4100])
            fpsfT = wqf[:, 0:4]
            wqT = wqf[:, 4:4100]
            wkbd = wpool.tile([128, 4096], BF16, tag="wkbd")
            nc.sync.dma_start(wkbd[:, 0:2048], wkbd_d.ap()[:, 0:2048])
            nc.sync.dma_start(wkbd[:, 2048:4096], wkbd_d.ap()[:, 2048:4096])
            imgs = []
            for q in range(2):
                t = ipool.tile([128, JS], BF16, tag=f"img{q}")
                nc.sync.dma_start(t[:], fimg_d.ap()[128 * q:128 * (q + 1), :])
                imgs.append(t)

            # A: q2T[p_row, 4p+b] = q2[b, 128p + p_row] (scale folded
            # into the PSUM->SBUF copy below)
            q2T_ps = spsum.tile([128, 128], F32, tag="q2T_ps")
            for p in range(32):
                nc.tensor.matmul(
                    q2T_ps[:, 4 * p:4 * p + 4],
                    wqT[:, 128 * p:128 * p + 128],
                    fpsfT,
                    start=True, stop=True,
                )
            q2T = wpool.tile([128, 128], BF16, tag="q2T")
            nc.vector.tensor_scalar_mul(q2T[:], q2T_ps[:], SCALE)

            # B: w2[c + 64*(h%2), 4*(h//2)+b] = W2[b, h, c] (scaled)
            w2_ps = spsum.tile([128, 128], F32, tag="w2_ps")
            for p in range(32):
                nc.tensor.matmul(
                    w2_ps[:, 4 * p:4 * p + 4],
                    wkbd[:, 128 * p:128 * p + 128],
                    q2T[:, 4 * p:4 * p + 4],
                    start=True, stop=True,
                )

            # Assembly: bd_q[64*half + c, 64*half + h] = W2[2q+half, h, c]
            bds = []
            for q in range(2):
                bd = wpool.tile([128, 128], BF16, tag=f"bd{q}")
                nc.vector.memset(bd[:], 0.0)
                for half in range(2):
                    b = 2 * q + half
                    for parity in range(2):
                        dst = bd[64 * half:64 * half + 64,
                                 64 * half + parity:64 * half + 64:2]
                        src = w2_ps[64 * parity:64 * parity + 64, b:128:4]
                        nc.vector.tensor_copy(dst, src)
                bds.append(bd)

            # Big: out rows pair q = bd_q.T @ img_q, in 512-col chunks.
            # Stage two chunks per [128, 1024] tile -> 4 output DMAs
            # instead of 8 (dma_start issue costs ~0.6us each on sync).
            for q in range(2):
                for half in range(2):
                    ot = opool.tile([128, 1024], F32, tag="ot")
                    for kk in range(2):
                        k = 2 * half + kk
                        ps = bpsum.tile([128, 512], F32, tag="mm_ps")
                        nc.tensor.matmul(
                            ps[:], bds[q][:],
                            imgs[q][:, 512 * k:512 * k + 512],
                            start=True, stop=True,
                        )
                        dst = ot[:, 512 * kk:512 * kk + 512]
                        if k % 2 == 0:
                            nc.vector.tensor_copy(dst, ps[:])
                        else:
                            nc.scalar.copy(dst, ps[:])
                    nc.sync.dma_start(
                        out_d.ap()[128 * q:128 * (q + 1),
                                   1024 * half:1024 * half + 1024],
                        ot[:],
                    )

    nc.compile()
    return nc


def _prep_inputs(fpsf, fimg, Wq, Wk):
    fpsf = np.ascontiguousarray(fpsf, dtype=np.float32)
    fimg = np.ascontiguousarray(fimg, dtype=np.float32)
    Wq = np.ascontiguousarray(Wq, dtype=np.float32)
    Wk = np.ascontiguousarray(Wk, dtype=np.float32)

    WqF = np.empty((64, 4100), NPBF16)
    WqF[:, 0:4] = fpsf.T.astype(NPBF16)
    WqF[:, 4:4100] = Wq.T.astype(NPBF16)

    Wk3 = Wk.reshape(64, 64, 64)  # [h, d, c]
    bd = np.zeros((128, 32, 128), np.float32)
    bd[0:64, :, 0:64] = Wk3[0::2].transpose(1, 0, 2)   # [d, pair, c]
    bd[64:128, :, 64:128] = Wk3[1::2].transpose(1, 0, 2)
    Wk_bd = np.ascontiguousarray(bd.reshape(128, 4096)).astype(NPBF16)

    fimg_f = fimg.reshape(B, C, HW).astype(NPBF16)
    in_maps = []
    for i in range(N_CORES):
        sh = np.ascontiguousarray(
            fimg_f[:, :, JS * i:JS * (i + 1)]).reshape(2 * 128, JS)
        in_maps.append({
            "fimg_s": sh,
            "WqF": WqF,
            "Wk_bd": Wk_bd,
        })
    return in_maps


def kernel(fpsf, fimg, Wq, Wk):
    global _compiled
    if _compiled is None:
        _compiled = _build()
    nc = _compiled

    in_maps = _prep_inputs(fpsf, fimg, Wq, Wk)
    res = run_bass_kernel_spmd(nc, in_maps, core_ids=list(range(N_CORES)))

    out = np.empty((B, HEADS, HW), dtype=np.float32)
    for i in range(N_CORES):
        out[:, :, JS * i:JS * (i + 1)] = \
            res.results[i]["out"].reshape(B, HEADS, JS)
    return out.reshape(B, C, H, W)


if __name__ == "__main__":
    rng = np.random.default_rng(0)
    ins = {
        "fpsf": rng.standard_normal((B, C), dtype=np.float32),
        "fimg": rng.standard_normal((B, C, H, W), dtype=np.float32),
        "Wq": (rng.standard_normal((4096, C), dtype=np.float32) * 0.05),
        "Wk": (rng.standard_normal((4096, C), dtype=np.float32) * 0.05),
    }
    out = kernel(**ins)
    print("out", out.shape, out.dtype, float(np.abs(out).max()))



# revision 3
# speedup vs baseline: 1.1724x; 1.1724x over previous
"""Trainium2 Bass kernel for nn_CrossAttention (single-query cross attention).

Reference computation (B=4, C=64, H=W=128, heads h=64, dim_head d=64,
inner=4096, HW=16384):
    x[b, j, c]   = fimg[b, c, j]                       (j indexes H*W)
    q[b, h, d]   = sum_e fpsf[b, e] Wq[h*64+d, e]
    k[b, j, h, d]= sum_c x[b, j, c] Wk[h*64+d, c]
    out[b, h, j] = scale * sum_d q[b,h,d] k[b,j,h,d]

Single query per (batch, head) -> the attention collapses:
    W2[b, h, c]  = scale * sum_d q[b,h,d] Wk[h*64+d, c]      (tiny)
    out[b, h, j] = sum_c W2[b,h,c] fimg[b, c, j]

Sharding: j (H*W = 16384) split across 8 cores (2048 each). Every core
redundantly computes W2 (needs all heads for its output slice).

The kernel is DMA-stream bound: per core it moves ~1MB weights + 1MB img
in and 1MB out (everything bf16; host casts back to f32 = layout only).
Design notes (from trace analysis of the previous 29.4us version):
  - HWDGE dma_starts issued from nc.sync execute FIFO on one ring, so
    descriptors are issued in exact compute order (wq0, wk0, wq1, wk1,
    img0, img1) and the PE pipeline (A half -> B half -> assembly) hides
    entirely under the img transfer.
  - Wk is sent DENSE [64, 4096] (d-major per head); step B runs 64
    per-head [64x64] matmuls with all operands at partition base 0 -- no
    block-diagonal tile, saving 512KB of zero DMA.
  - Output staged to SBUF as bf16 (halves output bytes) and written with
    4 DMAs of [128, 1024] so the first transfer starts ~2us before the
    last chunk is staged.

Device layouts (host does LAYOUT/dtype-cast only, no math):
  wq   [64, 4100] bf16: cols 0:4 = fpsf.T, cols 4: = Wq.T
  wk   [64, 4096] bf16: wk[d, 64h+c] = Wk[64h+d, c]
  img  [128, 4096] bf16: rows 64*(b%2)+c, cols 2048*(b//2)+j_local
  out  [128, 4096] bf16: rows 64*(b%2)+h, cols 2048*(b//2)+j_local

Device compute per core:
  A: 32 matmuls  q2_ps[128, 4p:4p+4] = wqT_chunk.T @ fpsfT
     (rows of chunk p: inner 128p+r -> head 2p + r//64, d = r%64)
  q2T [64, 256] bf16 = scale * q2_ps, 4 contiguous quadrant copies:
     q2T[d, 128*par + 4p+b] = q2_ps[64*par + d, 4p+b]
  B: 64 matmuls  w2_ps[0:64, 4h:4h+4] = wk[:, 64h:64h+64].T @ q2T[:, ...]
     -> w2_ps[c, 4h+b] = W2[b, h, c] (scaled)
  Assembly: per batch b (q=b//2, half=b%2), one [64,64] stride-4 copy:
     bd_q[64*half + c, 64*half + h] = w2_ps[c, 4h+b]   (bd memset early)
  Big: 8 matmuls [128, 512] = bd_q.T @ img chunk; psum -> bf16 SBUF
     staging (vector/scalar alternate); 4 output DMAs of [128, 1024].
"""

import sys
import types

import numpy as np
import ml_dtypes

# antenv.axon_hooks is absent in this image; bass_utils imports it when
# tracing. Register a minimal stand-in before importing concourse.
if "antenv.axon_hooks" not in sys.modules:
    try:
        import antenv  # noqa: F401

        _hooks = types.ModuleType("antenv.axon_hooks")
        _hooks._hook = None

        def _set_hook(h):
            _hooks._hook = h

        _hooks.set_axon_ntff_profile_hook = _set_hook
        _hooks.get_axon_ntff_profile_hook = lambda: _hooks._hook
        sys.modules["antenv.axon_hooks"] = _hooks
        try:
            from trn_agent_boot.trn_boot import _ntff_profile_via_ctypes

            _set_hook(_ntff_profile_via_ctypes("/opt/axon/libaxon_pjrt.so"))
        except Exception:
            pass
    except ImportError:
        pass

import concourse.bass as bass  # noqa: E402
import concourse.mybir as mybir  # noqa: E402
import concourse.tile as tile  # noqa: E402
from concourse import bacc  # noqa: E402
from concourse.bass_utils import run_bass_kernel_spmd  # noqa: E402

N_CORES = 8
B, C, H, W = 4, 64, 128, 128
HEADS, DIM_HEAD = 64, 64
HW = H * W
JS = HW // N_CORES  # 2048 j-positions per core
SCALE = DIM_HEAD ** -0.5
F32 = mybir.dt.float32
BF16 = mybir.dt.bfloat16
NPBF16 = ml_dtypes.bfloat16

_compiled = None  # cache (nc) across calls


def _build():
    nc = bacc.Bacc("TRN2", target_bir_lowering=False, debug=False,
                   num_devices=N_CORES)

    wq_d = nc.dram_tensor("wq", [64, 4100], BF16, kind="ExternalInput")
    wk_d = nc.dram_tensor("wk", [64, 4096], BF16, kind="ExternalInput")
    img_d = nc.dram_tensor("img", [128, 2 * JS], BF16, kind="ExternalInput")
    out_d = nc.dram_tensor("out", [128, 2 * JS], BF16, kind="ExternalOutput")

    with tile.TileContext(nc) as tc:
        with (
            tc.tile_pool(name="weights", bufs=1) as wpool,
            tc.tile_pool(name="img", bufs=1) as ipool,
            tc.tile_pool(name="small_ps", bufs=1, space="PSUM") as spsum,
            tc.tile_pool(name="big_ps", bufs=4, space="PSUM") as bpsum,
            tc.tile_pool(name="ostage", bufs=2) as opool,
        ):
            # bd tiles zeroed first thing -- vector is idle until the
            # weights land, so the memsets are free.
            bds = []
            for q in range(2):
                bd = wpool.tile([128, 128], BF16, tag=f"bd{q}")
                nc.vector.memset(bd[:], 0.0)
                bds.append(bd)

            # Input DMAs in compute order on one FIFO ring (sync HWDGE):
            # wq half 0 (with fpsf), wk half 0, wq half 1, wk half 1,
            # img pair 0, img pair 1.
            wq = wpool.tile([64, 4100], BF16, tag="wq")
            wk = wpool.tile([64, 4096], BF16, tag="wk")
            imgs = [ipool.tile([128, JS], BF16, tag=f"img{q}", name=f"img{q}")
                    for q in range(2)]
            nc.sync.dma_start(wq[:, 0:2052], wq_d.ap()[:, 0:2052])
            nc.sync.dma_start(wk[:, 0:2048], wk_d.ap()[:, 0:2048])
            nc.sync.dma_start(wq[:, 2052:4100], wq_d.ap()[:, 2052:4100])
            nc.sync.dma_start(wk[:, 2048:4096], wk_d.ap()[:, 2048:4096])
            for q in range(2):
                nc.sync.dma_start(imgs[q][:], img_d.ap()[:, JS * q:JS * (q + 1)])

            fpsfT = wq[:, 0:4]
            q2_ps = spsum.tile([128, 128], F32, tag="q2_ps")
            w2_ps = spsum.tile([64, 256], F32, tag="w2_ps")
            q2T = wpool.tile([64, 256], BF16, tag="q2T")

            # Two half-pipelines: A chunk -> q copies -> B heads, so step
            # B for heads 0-31 runs while wq half 1 / wk half 1 stream in.
            for ph in range(2):
                # A: q2_ps[r, 4p+b] = q2[b, 128p+r]
                for p in range(16 * ph, 16 * ph + 16):
                    nc.tensor.matmul(
                        q2_ps[:, 4 * p:4 * p + 4],
                        wq[:, 4 + 128 * p:4 + 128 * p + 128],
                        fpsfT,
                        start=True, stop=True,
                    )
                # scale folded into the PSUM->SBUF quadrant copies
                for par in range(2):
                    nc.vector.tensor_scalar_mul(
                        q2T[:, 128 * par + 64 * ph:128 * par + 64 * ph + 64],
                        q2_ps[64 * par:64 * par + 64, 64 * ph:64 * ph + 64],
                        SCALE,
                    )
                # B: w2_ps[c, 4h+b] = W2[b, h, c]
                for h in range(32 * ph, 32 * ph + 32):
                    p, par = h // 2, h % 2
                    nc.tensor.matmul(
                        w2_ps[:, 4 * h:4 * h + 4],
                        wk[:, 64 * h:64 * h + 64],
                        q2T[:, 128 * par + 4 * p:128 * par + 4 * p + 4],
                        start=True, stop=True,
                    )

            # Assembly: bd_q[64*half + c, 64*half + h] = w2_ps[c, 4h+b]
            for b in range(4):
                q, half = b // 2, b % 2
                nc.vector.tensor_copy(
                    bds[q][64 * half:64 * half + 64,
                           64 * half:64 * half + 64],
                    w2_ps[:, b:256:4],
                )

            # Big: out rows pair q = bd_q.T @ img_q, 512-col chunks into
            # a [128, 2048] bf16 staging tile; output DMA per 1024 cols.
            for q in range(2):
                ot = opool.tile([128, JS], BF16, tag="ot")
                for k in range(4):
                    ps = bpsum.tile([128, 512], F32, tag="mm_ps")
                    nc.tensor.matmul(
                        ps[:], bds[q][:],
                        imgs[q][:, 512 * k:512 * k + 512],
                        start=True, stop=True,
                    )
                    dst = ot[:, 512 * k:512 * k + 512]
                    if k % 2 == 0:
                        nc.vector.tensor_copy(dst, ps[:])
                    else:
                        nc.scalar.copy(dst, ps[:])
                    if k % 2 == 1:
                        nc.sync.dma_start(
                            out_d.ap()[:, JS * q + 512 * (k - 1):
                                       JS * q + 512 * (k + 1)],
                            ot[:, 512 * (k - 1):512 * (k + 1)],
                        )

    nc.compile()
    return nc


def _prep_inputs(fpsf, fimg, Wq, Wk):
    fpsf = np.ascontiguousarray(fpsf, dtype=np.float32)
    fimg = np.ascontiguousarray(fimg, dtype=np.float32)
    Wq = np.ascontiguousarray(Wq, dtype=np.float32)
    Wk = np.ascontiguousarray(Wk, dtype=np.float32)

    wq = np.empty((64, 4100), NPBF16)
    wq[:, 0:4] = fpsf.T.astype(NPBF16)
    wq[:, 4:4100] = Wq.T.astype(NPBF16)

    # wk[d, 64h+c] = Wk[64h+d, c]
    wk = np.ascontiguousarray(
        Wk.reshape(64, 64, 64).transpose(1, 0, 2).reshape(64, 4096)
    ).astype(NPBF16)

    fimg_f = fimg.reshape(B, C, HW).astype(NPBF16)
    in_maps = []
    for i in range(N_CORES):
        sh = fimg_f[:, :, JS * i:JS * (i + 1)]  # [4, 64, JS]
        # rows 64*(b%2)+c, cols JS*(b//2)+j
        img = np.ascontiguousarray(
            sh.reshape(2, 2, 64, JS).transpose(1, 2, 0, 3).reshape(128, 2 * JS)
        )
        in_maps.append({"wq": wq, "wk": wk, "img": img})
    return in_maps


def kernel(fpsf, fimg, Wq, Wk):
    global _compiled
    if _compiled is None:
        _compiled = _build()
    nc = _compiled

    in_maps = _prep_inputs(fpsf, fimg, Wq, Wk)
    res = run_bass_kernel_spmd(nc, in_maps, core_ids=list(range(N_CORES)))

    out = np.empty((B, HEADS, HW), dtype=np.float32)
    for i in range(N_CORES):
        r = res.results[i]["out"]  # [128, 2*JS] bf16
        out[:, :, JS * i:JS * (i + 1)] = (
            np.asarray(r).reshape(2, 64, 2, JS).transpose(2, 0, 1, 3)
            .reshape(B, HEADS, JS).astype(np.float32)
        )
    return out.reshape(B, C, H, W)


if __name__ == "__main__":
    rng = np.random.default_rng(0)
    ins = {
        "fpsf": rng.standard_normal((B, C), dtype=np.float32),
        "fimg": rng.standard_normal((B, C, H, W), dtype=np.float32),
        "Wq": (rng.standard_normal((4096, C), dtype=np.float32) * 0.05),
        "Wk": (rng.standard_normal((4096, C), dtype=np.float32) * 0.05),
    }
    out = kernel(**ins)
    print("out", out.shape, out.dtype, float(np.abs(out).max()))
